# revision 14
# baseline (speedup 1.0000x reference)
"""MoE pre-activation residual block on 8 trn2 NeuronCores (expert-parallel).

kernel(**inputs) takes the full unsharded inputs (numpy, keyed as in
setup_inputs) and returns the full [N, D] float32 output.

Host: LayerNorm+relu, router logits, top-2 gating, capacity-based dispatch
      (builds expert_inputs per expert), final gather/combine/residual.
Device (one expert per core, SPMD): the expert MLP
      y = LN_h(x @ W1 + b1) -> relu -> @ W2 + b2
  computed as two fp8(e4m3) DoubleRow matmuls with fp32 PSUM accumulation
  (each matmul instruction contracts 256 = 2x128 via paired k-subtiles):
    - h^T[H, c] = sum_d W1'[d, h] x^T[d, c]  (lhsT = W1 as stored); the LN
      mean over H is folded into the weights on the host
      (W1' = (W1 - rowmean_H(W1)) * 2^S1), so PSUM holds 2^S1 (h - mu)
    - var = sum_H (h-mu)^2: ACT squares each PSUM tile (bf16), DVE folds the
      16 tiles with a pairwise add tree, PE does ONE ones-matmul reduction;
      the 2^S1 / 2^S2 prescales fold exactly into the Sqrt bias/scale
    - hn = relu(2^S1 (h - mu)) cast to fp8 straight from PSUM (rstd > 0
      commutes w/ relu, so it can be applied after mm2)
    - y^T[D, c] = sum_h (2^S2 W2[h, d]) hn[h, c]; rb = rstd 2^-(S1+S2)
      (broadcast across partitions by GpSimd) applied at PSUM eviction on DVE
"""

import sys

try:
    import concourse.bacc  # noqa: F401
except ImportError:  # pragma: no cover
    for _p in ("/opt/trn_rl_repo", "/root/.axon_site/_ro/trn_rl_repo"):
        if _p not in sys.path:
            sys.path.append(_p)

import numpy as np
import ml_dtypes

import concourse.bacc as bacc
import concourse.mybir as mybir
import concourse.tile as tile
from concourse.bass_utils import run_bass_kernel_spmd

# ---------------------------------------------------------------- shim -----
# Under axon, run_bass_kernel_spmd(trace=True) needs antenv.axon_hooks for
# NTFF profiling. Some images lack it; register an equivalent hook so a
# BASS_TRACE=1 run still produces timing instead of silently skipping.
def _install_axon_hooks_shim():
    try:
        import antenv.axon_hooks  # noqa: F401
        return
    except ImportError:
        pass
    import contextlib, ctypes, types, os

    so = "/opt/axon/libaxon_pjrt.so"
    hook = None
    if os.path.exists(so):
        try:
            lib = ctypes.CDLL(so)
            if hasattr(lib, "axon_start_nrt_profile"):
                lib.axon_start_nrt_profile.argtypes = [
                    ctypes.POINTER(ctypes.c_int64),
                    ctypes.c_size_t,
                ]
                lib.axon_start_nrt_profile.restype = ctypes.c_int64
                lib.axon_stop_nrt_profile.argtypes = [ctypes.c_char_p]
                lib.axon_stop_nrt_profile.restype = ctypes.c_int64

                @contextlib.contextmanager
                def _hook(output_dir, device_ids):
                    import jax

                    jax.devices()
                    if device_ids:
                        ids = (ctypes.c_int64 * len(device_ids))(*device_ids)
                        rc = lib.axon_start_nrt_profile(ids, len(device_ids))
                    else:
                        rc = lib.axon_start_nrt_profile(None, 0)
                    if rc != 0:
                        raise RuntimeError(f"axon_start_nrt_profile rc={rc}")
                    try:
                        yield
                    finally:
                        n = lib.axon_stop_nrt_profile(str(output_dir).encode())
                        print(f"ntff profile: {n} file(s) -> {output_dir}",
                              file=sys.stderr)

                hook = _hook
        except OSError:
            hook = None
    mod = types.ModuleType("antenv.axon_hooks")
    mod.get_axon_ntff_profile_hook = lambda: hook
    mod.set_axon_ntff_profile_hook = lambda h: None
    sys.modules["antenv.axon_hooks"] = mod


_install_axon_hooks_shim()

# ------------------------------------------------------------- constants ---
N, D, H, E, TOPK = 16384, 1024, 2048, 8, 2
CAP = 4096
EPS = 1e-6
P = 128
C = 512                      # CAP-chunk (columns per pipeline step)
KD, KH = D // P, H // P      # 8 k-subtiles for mm1, 16 for mm2
MT = H // P                  # 16 output row-tiles of mm1 (H rows)
DT = D // P                  # 8 output row-tiles of mm2 (D rows)
NCH = CAP // C               # chunks
S1, S2 = 4, 5                # power-of-2 prescales on W1', W2 (fp8 range fit)

BF16 = mybir.dt.bfloat16
FP8 = mybir.dt.float8e4
F32 = mybir.dt.float32
DR = mybir.MatmulPerfMode.DoubleRow
npbf16 = ml_dtypes.bfloat16
npfp8 = ml_dtypes.float8_e4m3

_nc_cache = {}


def _build(flags):
    """Build the per-core SPMD bass program. flags = (b1_nz, ns_nb_nz, b2_nz).

    The LayerNorm mean over H is folded into the weights on the host
    (W1' = (W1 - rowmean_H(W1)) * 2^S1, b1' = (b1 - mean(b1)) * 2^S1), so
    PSUM holds 2^S1 (h - mu) directly after the W1' matmul."""
    b1_nz, ns_nb_nz, b2_nz = flags
    nc = bacc.Bacc("TRN2", target_bir_lowering=False)

    xT_d = nc.dram_tensor("xT", [D, CAP], FP8, kind="ExternalInput")
    w1_d = nc.dram_tensor("w1", [D, H], FP8, kind="ExternalInput")
    w2_d = nc.dram_tensor("w2", [H, D], FP8, kind="ExternalInput")
    yT_d = nc.dram_tensor("yT", [D, CAP], F32, kind="ExternalOutput")
    if not ns_nb_nz:
        # fast path: rstd is a per-slot scalar; ship std to the host, which
        # folds 1/std (and b2) into the combine weights
        std_d = nc.dram_tensor("stdr", [1, CAP], F32, kind="ExternalOutput")
    if b1_nz:
        b1_d = nc.dram_tensor("b1", [H, 1], BF16, kind="ExternalInput")
    if ns_nb_nz:
        nsc_d = nc.dram_tensor("nsc", [H, 1], F32, kind="ExternalInput")
        nbs_d = nc.dram_tensor("nbs", [H, 1], F32, kind="ExternalInput")
    if b2_nz and ns_nb_nz:
        b2_d = nc.dram_tensor("b2", [D, 1], F32, kind="ExternalInput")

    xT_r = xT_d.rearrange("(ko p) c -> p ko c", p=P)
    w1_r = w1_d.rearrange("(ko p) h -> p ko h", p=P)
    w2_r = w2_d.rearrange("(ko p) d -> p ko d", p=P)
    yT_r = yT_d.rearrange("(dt p) c -> p dt c", p=P)

    with tile.TileContext(nc) as tc:
        with (
            tc.tile_pool(name="const", bufs=1) as cpool,
            tc.tile_pool(name="xp", bufs=3) as xpool,
            tc.tile_pool(name="hnp", bufs=2) as hnpool,
            tc.tile_pool(name="sqp", bufs=4) as sqpool,
            tc.tile_pool(name="rows", bufs=3) as rowpool,
            tc.tile_pool(name="rbp", bufs=2) as rbpool,
            tc.tile_pool(name="yp", bufs=3) as ypool,
            tc.tile_pool(name="hgen", bufs=2) as hgenpool,
            tc.tile_pool(name="ps_h", bufs=3, space="PSUM") as ps_h,
            tc.tile_pool(name="ps_y", bufs=4, space="PSUM") as ps_y,
            tc.tile_pool(name="ps_s", bufs=1, space="PSUM") as ps_s,
        ):
            # ---- resident constants (x chunk 0 first, then W1 in row-tile
            # slices so the first matmul group can start after ~0.7MB of DMA,
            # W2 last: not needed until the first mm2) ----------------------
            x_tiles = [None] * NCH

            def emit_x_load(c, split=False):
                x_tiles[c] = xpool.tile([P, KD, C], FP8, tag="x", name="x")
                if split:
                    # chunk 0 gates the first matmul: split each ko slice
                    # into partition halves so no single DMA queue serializes
                    # more than 64 descriptors before compute can start
                    for kt in range(KD):
                        for p0 in (0, 64):
                            nc.sync.dma_start(
                                x_tiles[c][p0:p0 + 64, kt, :],
                                xT_r[p0:p0 + 64, kt, c * C:(c + 1) * C],
                            )
                else:
                    nc.sync.dma_start(x_tiles[c][:], xT_r[:, :, c * C:(c + 1) * C])

            # interleave the first x chunk with the first W1 row-tile slices
            # so the first matmul group's inputs land as early as possible
            w1_sb = cpool.tile([P, KD, H], FP8, tag="w1", name="w1")
            nc.sync.dma_start(w1_sb[:, :, 0:P], w1_r[:, :, 0:P])
            emit_x_load(0, split=True)
            for mt in range(1, MT):
                nc.sync.dma_start(
                    w1_sb[:, :, mt * P:(mt + 1) * P], w1_r[:, :, mt * P:(mt + 1) * P]
                )
            ones_kcol = cpool.tile([P, 1], BF16, tag="ones_kcol", name="ones_kcol")
            nc.vector.memset(ones_kcol[:], 1.0)
            eps_sb = cpool.tile([1, 1], F32, tag="eps", name="eps")
            # Sqrt bias absorbs the power-of-2 prescales (see module docstring)
            if ns_nb_nz:
                nc.vector.memset(eps_sb[:], EPS * 2.0 ** (2 * S1))
            else:
                nc.vector.memset(eps_sb[:], EPS * 2.0 ** (2 * (S1 + S2)))
            if b1_nz:
                b1_sb = cpool.tile([1, H], BF16, tag="b1", name="b1")
                nc.sync.dma_start(b1_sb[:], b1_d.rearrange("h x -> x h"))
                ones_row = cpool.tile([1, C], BF16, tag="ones_row", name="ones_row")
                nc.vector.memset(ones_row[:], 1.0)
            if ns_nb_nz:
                nsc_sb = cpool.tile([P, MT], F32, tag="nsc", name="nsc")
                nc.sync.dma_start(nsc_sb[:], nsc_d.rearrange("(mt p) x -> p mt x", p=P)[:, :, 0])
                nbs_sb = cpool.tile([P, MT], F32, tag="nbs", name="nbs")
                nc.sync.dma_start(nbs_sb[:], nbs_d.rearrange("(mt p) x -> p mt x", p=P)[:, :, 0])
            if b2_nz and ns_nb_nz:
                b2_sb = cpool.tile([P, DT], F32, tag="b2", name="b2")
                nc.sync.dma_start(b2_sb[:], b2_d.rearrange("(dt p) x -> p dt x", p=P)[:, :, 0])
            w2_sb = cpool.tile([P, KH, D], FP8, tag="w2", name="w2")
            for kt in range(KH):
                nc.sync.dma_start(w2_sb[:, kt, :], w2_r[:, kt, :])

            for c in range(NCH):
                xt = x_tiles[c]
                hn = hnpool.tile([P, KH, C], FP8, tag="hn", name="hn")
                hflat = hgenpool.tile([P, KH, C], F32, tag="hflat", name="hflat") if ns_nb_nz else None
                # mm1: 16 row-tile groups of 4 DoubleRow matmuls (256
                # contraction each). ACT squares each PSUM tile (bf16), DVE
                # folds the squared tiles with a linear running sum (one add
                # per group window; after the LAST square only one add
                # remains, so the stats chain launches ~1us after the last
                # mm1 matmul), then PE does a single ones-matmul reduction.
                acc = None
                for mt in range(MT):
                    ph = ps_h.tile([P, C], F32, tag="ph", name="ph")
                    for kt in range(0, KD, 2):
                        nc.tensor.matmul(
                            ph[:], lhsT=w1_sb[:, kt:kt + 2, mt * P:(mt + 1) * P],
                            rhs=xt[:, kt:kt + 2, :], start=(kt == 0),
                            stop=(kt == KD - 2 and not b1_nz), perf_mode=DR,
                        )
                    if b1_nz:
                        nc.tensor.matmul(
                            ph[:], lhsT=b1_sb[:, mt * P:(mt + 1) * P], rhs=ones_row[:],
                            start=False, stop=True, skip_group_check=True,
                        )
                    sq = sqpool.tile([P, C], BF16, tag="sq", name="sq")
                    nc.scalar.square(sq[:], ph[:])
                    if ns_nb_nz:
                        nc.vector.tensor_copy(hflat[:, mt, :], ph[:])
                    else:
                        nc.vector.tensor_scalar_max(hn[:, mt, :], ph[:], 0.0)
                    if acc is None:
                        acc = sq
                    else:
                        nxt = sqpool.tile([P, C], BF16, tag=f"acc{mt % 2}",
                                          name="acc")
                        nc.vector.tensor_add(nxt[:], acc[:], sq[:])
                        acc = nxt
                hacc_bf = acc

                if c + 1 < NCH:
                    emit_x_load(c + 1)

                def emit_stats_head(ss, sqrt_scale):
                    # ss[1, C] = sum_p hacc_bf -> std -> rstd (row ops). The
                    # [1, C] DVE reciprocal costs ~3.2us; the whole chain is
                    # issued before the mm2 groups so 4 PSUM y-banks of PE
                    # run-ahead absorb it.
                    nc.tensor.matmul(ss[:1, :], lhsT=ones_kcol[:], rhs=hacc_bf[:],
                                     start=True, stop=True, skip_group_check=True)
                    std = rowpool.tile([1, C], F32, tag="std", name="std")
                    nc.scalar.activation(
                        std[:], ss[:1, :], mybir.ActivationFunctionType.Sqrt,
                        bias=eps_sb[:], scale=sqrt_scale,
                    )
                    rstd = rowpool.tile([1, C], F32, tag="rstd", name="rstd")
                    nc.vector.reciprocal(rstd[:], std[:])
                    return rstd

                def emit_rb(rstd):
                    # broadcast rstd across partitions on the (idle) GpSimd
                    rb = rbpool.tile([P, C], F32, tag="rb", name="rb")
                    nc.gpsimd.partition_broadcast(rb[:], rstd[:], channels=P)
                    return rb

                if ns_nb_nz:
                    # general path: hn = relu(((h-mu)*rstd)*nsc + nbs);
                    # rb here = rstd 2^-S1, and the 2^-S2 is applied at the
                    # mm2 eviction below.
                    ss = ps_s.tile([P, C], F32, tag="small", name="small")
                    rstd = emit_stats_head(ss, 1.0 / H)
                    rb = emit_rb(rstd)
                    for mt in range(MT):
                        tmp = hgenpool.tile([P, C], F32, tag="tmpn", name="tmpn")
                        nc.vector.tensor_mul(tmp[:], hflat[:, mt, :], rb[:])
                        nc.scalar.activation(
                            hn[:, mt, :], tmp[:],
                            mybir.ActivationFunctionType.Relu,
                            bias=nbs_sb[:, mt, None], scale=nsc_sb[:, mt, None],
                        )

                    for dt in range(DT):
                        py = ps_y.tile([P, C], F32, tag="py", name="py")
                        for kt in range(0, KH, 2):
                            nc.tensor.matmul(
                                py[:], lhsT=w2_sb[:, kt:kt + 2, dt * P:(dt + 1) * P],
                                rhs=hn[:, kt:kt + 2, :], start=(kt == 0),
                                stop=(kt == KH - 2), perf_mode=DR,
                            )
                        ysb = ypool.tile([P, C], F32, tag="y", name="y")
                        nc.vector.tensor_scalar_mul(ysb[:], py[:], 2.0 ** (-S2))
                        if b2_nz:
                            nc.vector.tensor_scalar_add(ysb[:], ysb[:], b2_sb[:, dt, None])
                        nc.sync.dma_start(yT_r[:, dt, c * C:(c + 1) * C], ysb[:])
                else:
                    # fast path: the per-slot rstd (and b2) are applied on
                    # the HOST during combine, so mm2 evictions are plain ACT
                    # copies with no dependency on the stats chain. The
                    # device only ships std = sqrt(ss*2^(2S2)/H + eps') rows.
                    pys = [None] * DT

                    def y_mms(dt):
                        pys[dt] = ps_y.tile([P, C], F32, tag="py", name="py")
                        for kt in range(0, KH, 2):
                            nc.tensor.matmul(
                                pys[dt][:], lhsT=w2_sb[:, kt:kt + 2, dt * P:(dt + 1) * P],
                                rhs=hn[:, kt:kt + 2, :], start=(kt == 0),
                                stop=(kt == KH - 2), perf_mode=DR,
                            )

                    def y_evict(dt):
                        ysb = ypool.tile([P, C], F32, tag="y", name="y")
                        nc.scalar.activation(ysb[:], pys[dt][:],
                                             mybir.ActivationFunctionType.Copy)
                        if c == NCH - 1:
                            # the very last stores are exposed at the kernel
                            # tail: split across partition strips -> 4 queues
                            for p0 in range(0, P, 32):
                                nc.sync.dma_start(
                                    yT_r[p0:p0 + 32, dt, c * C:(c + 1) * C],
                                    ysb[p0:p0 + 32, :])
                        else:
                            nc.sync.dma_start(yT_r[:, dt, c * C:(c + 1) * C], ysb[:])

                    # stats ones-matmul sits in the PE's in-order stream:
                    # issue it after the first mm2 group so the PE never
                    # waits on the square/add chain draining
                    y_mms(0)
                    ss = ps_s.tile([P, C], F32, tag="small", name="small")
                    nc.tensor.matmul(ss[:1, :], lhsT=ones_kcol[:], rhs=hacc_bf[:],
                                     start=True, stop=True, skip_group_check=True)
                    std = rowpool.tile([1, C], F32, tag="std", name="std")
                    nc.scalar.activation(
                        std[:], ss[:1, :], mybir.ActivationFunctionType.Sqrt,
                        bias=eps_sb[:], scale=2.0 ** (2 * S2) / H,
                    )
                    nc.sync.dma_start(std_d[:, c * C:(c + 1) * C], std[:])
                    y_evict(0)
                    for dt in range(1, DT):
                        y_mms(dt)
                        y_evict(dt)

    nc.compile()
    return nc


# ------------------------------------------------------------ host logic ---
def _route(x0, ln_scale, ln_bias, Wr, br):
    """LayerNorm -> relu -> router logits -> top-2 -> gates (float64 math)."""
    x = x0.astype(np.float64)
    mu = x.mean(axis=-1, keepdims=True)
    var = np.square(x - mu).mean(axis=-1, keepdims=True)
    xn = (x - mu) / np.sqrt(var + EPS)
    xn = xn * ln_scale.astype(np.float64) + ln_bias.astype(np.float64)
    np.maximum(xn, 0.0, out=xn)
    logits = xn @ Wr.astype(np.float64) + br.astype(np.float64)

    n = logits.shape[0]
    rows = np.arange(n)
    i0 = np.argmax(logits, axis=1)
    l0 = logits[rows, i0]
    tmp = logits.copy()
    tmp[rows, i0] = -np.inf
    i1 = np.argmax(tmp, axis=1)
    l1 = tmp[rows, i1]
    # softmax over (l0, l1); l0 >= l1
    e1 = np.exp(l1 - l0)
    g0 = 1.0 / (1.0 + e1)
    g1 = e1 / (1.0 + e1)
    top_idx = np.stack([i0, i1], axis=1).astype(np.int64)
    gates = np.stack([g0, g1], axis=1)
    return xn.astype(np.float32), top_idx, gates


def _positions(top_idx):
    """Capacity positions: running per-expert count in token-major slot order."""
    eidx = top_idx.reshape(-1)
    nk = eidx.shape[0]
    oh = (eidx[:, None] == np.arange(E)[None, :]).astype(np.int64)
    pos = np.cumsum(oh, axis=0)[np.arange(nk), eidx] - 1
    mask = pos < CAP
    pos_c = np.minimum(pos, CAP - 1)
    return eidx, pos, pos_c, mask


def kernel(**inputs):
    x0 = np.asarray(inputs["x0"], np.float32)
    ln_scale = np.asarray(inputs["ln_scale"], np.float32)
    ln_bias = np.asarray(inputs["ln_bias"], np.float32)
    Wr = np.asarray(inputs["Wr"], np.float32)
    br = np.asarray(inputs["br"], np.float32)
    W1 = np.asarray(inputs["W1"], np.float32)
    b1 = np.asarray(inputs["b1"], np.float32)
    n_scale = np.asarray(inputs["n_scale"], np.float32)
    n_bias = np.asarray(inputs["n_bias"], np.float32)
    W2 = np.asarray(inputs["W2"], np.float32)
    b2 = np.asarray(inputs["b2"], np.float32)

    # ---- host routing + dispatch ---------------------------------------
    xn, top_idx, gates = _route(x0, ln_scale, ln_bias, Wr, br)
    eidx, pos, pos_c, mask = _positions(top_idx)

    tok_of_slot = np.repeat(np.arange(N), TOPK)
    keep = mask
    expert_inputs = np.zeros((E, CAP, D), np.float32)
    expert_inputs[eidx[keep], pos[keep]] = xn[tok_of_slot[keep]]

    # ---- build / fetch compiled program --------------------------------
    b1_nz = bool(np.any(b1))
    ns_nb_nz = bool(np.any(n_scale != 1.0) or np.any(n_bias))
    b2_nz = bool(np.any(b2))
    flags = (b1_nz, ns_nb_nz, b2_nz)
    if flags not in _nc_cache:
        _nc_cache[flags] = _build(flags)
    nc = _nc_cache[flags]

    # ---- per-core inputs ----------------------------------------------
    in_maps = []
    for e in range(E):
        # Fold the LayerNorm mean over H into the weights: x @ W1' = h - mu.
        w1p = W1[e].astype(np.float64)
        w1p = w1p - w1p.mean(axis=1, keepdims=True)
        m = {
            "xT": np.ascontiguousarray(expert_inputs[e].T).astype(npfp8),
            "w1": (w1p * 2.0 ** S1).astype(npfp8),
            "w2": (W2[e].astype(np.float64) * 2.0 ** S2).astype(npfp8),
        }
        if b1_nz:
            b1p = b1[e].astype(np.float64)
            b1p = b1p - b1p.mean()
            m["b1"] = (b1p * 2.0 ** S1).astype(npbf16)[:, None]
        if ns_nb_nz:
            m["nsc"] = n_scale[e].astype(np.float32)[:, None]
            m["nbs"] = n_bias[e].astype(np.float32)[:, None]
        if b2_nz and ns_nb_nz:
            m["b2"] = b2[e].astype(np.float32)[:, None]
        in_maps.append(m)

    res = run_bass_kernel_spmd(nc, in_maps, core_ids=list(range(E)))

    # ---- combine -------------------------------------------------------
    yT_all = np.stack([res.results[e]["yT"] for e in range(E)])  # [E, D, CAP]
    w = (gates.astype(np.float32) * mask.reshape(N, TOPK))
    pos2 = pos_c.reshape(N, TOPK)
    mix = np.zeros((N, D), np.float32)
    if ns_nb_nz:
        for k in range(TOPK):
            mix += yT_all[top_idx[:, k], :, pos2[:, k]] * w[:, k:k + 1]
    else:
        # fast path: device output is raw 2^(S1+S2) W2^T relu(h-mu); fold
        # rstd 2^-(S1+S2) = 1/std (and b2) into the per-slot combine weights
        std_all = np.stack([np.asarray(res.results[e]["stdr"], np.float64)
                            .reshape(CAP) for e in range(E)])  # [E, CAP]
        rb_all = 1.0 / std_all
        for k in range(TOPK):
            wk = (w[:, k] * rb_all[top_idx[:, k], pos2[:, k]]).astype(np.float32)
            mix += yT_all[top_idx[:, k], :, pos2[:, k]] * wk[:, None]
            if b2_nz:
                mix += b2[top_idx[:, k]] * w[:, k:k + 1]
    return x0 + mix


# revision 22
# speedup vs baseline: 1.2556x; 1.2556x over previous
"""MoE pre-activation residual block on 8 trn2 NeuronCores (expert-parallel).

kernel(**inputs) takes the full unsharded inputs (numpy, keyed as in
setup_inputs) and returns the full [N, D] float32 output.

Host: LayerNorm+relu, router logits, top-2 gating, capacity-based dispatch
      (builds expert_inputs per expert), final gather/combine/residual.
Device (one expert per core, SPMD): the expert MLP
      y = LN_h(x @ W1 + b1) -> relu -> @ W2 + b2
  computed as two fp8(e4m3) DoubleRow matmuls with fp32 PSUM accumulation
  (each matmul instruction contracts 256 = 2x128 via paired k-subtiles):
    - h^T[H, c] = sum_d W1'[d, h] x^T[d, c]  (lhsT = W1 as stored); the LN
      mean over H is folded into the weights on the host
      (W1' = (W1 - rowmean_H(W1)) * 2^S1), so PSUM holds 2^S1 (h - mu)
    - var = sum_H (h-mu)^2: ACT squares each PSUM tile (bf16), DVE folds the
      16 tiles with a pairwise add tree, PE does ONE ones-matmul reduction;
      the 2^S1 / 2^S2 prescales fold exactly into the Sqrt bias/scale
    - hn = relu(2^S1 (h - mu)) cast to fp8 straight from PSUM (rstd > 0
      commutes w/ relu, so it can be applied after mm2)
    - y^T[D, c] = sum_h (2^S2 W2[h, d]) hn[h, c]; rb = rstd 2^-(S1+S2)
      (broadcast across partitions by GpSimd) applied at PSUM eviction on DVE
"""

import sys

try:
    import concourse.bacc  # noqa: F401
except ImportError:  # pragma: no cover
    for _p in ("/opt/trn_rl_repo", "/root/.axon_site/_ro/trn_rl_repo"):
        if _p not in sys.path:
            sys.path.append(_p)

import numpy as np
import ml_dtypes

import concourse.bacc as bacc
import concourse.mybir as mybir
import concourse.tile as tile
from concourse.bass_utils import run_bass_kernel_spmd

# ---------------------------------------------------------------- shim -----
# Under axon, run_bass_kernel_spmd(trace=True) needs antenv.axon_hooks for
# NTFF profiling. Some images lack it; register an equivalent hook so a
# BASS_TRACE=1 run still produces timing instead of silently skipping.
def _install_axon_hooks_shim():
    try:
        import antenv.axon_hooks  # noqa: F401
        return
    except ImportError:
        pass
    import contextlib, ctypes, types, os

    so = "/opt/axon/libaxon_pjrt.so"
    hook = None
    if os.path.exists(so):
        try:
            lib = ctypes.CDLL(so)
            if hasattr(lib, "axon_start_nrt_profile"):
                lib.axon_start_nrt_profile.argtypes = [
                    ctypes.POINTER(ctypes.c_int64),
                    ctypes.c_size_t,
                ]
                lib.axon_start_nrt_profile.restype = ctypes.c_int64
                lib.axon_stop_nrt_profile.argtypes = [ctypes.c_char_p]
                lib.axon_stop_nrt_profile.restype = ctypes.c_int64

                @contextlib.contextmanager
                def _hook(output_dir, device_ids):
                    import jax

                    jax.devices()
                    if device_ids:
                        ids = (ctypes.c_int64 * len(device_ids))(*device_ids)
                        rc = lib.axon_start_nrt_profile(ids, len(device_ids))
                    else:
                        rc = lib.axon_start_nrt_profile(None, 0)
                    if rc != 0:
                        raise RuntimeError(f"axon_start_nrt_profile rc={rc}")
                    try:
                        yield
                    finally:
                        n = lib.axon_stop_nrt_profile(str(output_dir).encode())
                        print(f"ntff profile: {n} file(s) -> {output_dir}",
                              file=sys.stderr)

                hook = _hook
        except OSError:
            hook = None
    mod = types.ModuleType("antenv.axon_hooks")
    mod.get_axon_ntff_profile_hook = lambda: hook
    mod.set_axon_ntff_profile_hook = lambda h: None
    sys.modules["antenv.axon_hooks"] = mod


_install_axon_hooks_shim()

# ------------------------------------------------------------- constants ---
N, D, H, E, TOPK = 16384, 1024, 2048, 8, 2
CAP = 4096
EPS = 1e-6
P = 128
C = 512                      # CAP-chunk (columns per pipeline step)
KD, KH = D // P, H // P      # 8 k-subtiles for mm1, 16 for mm2
MT = H // P                  # 16 output row-tiles of mm1 (H rows)
DT = D // P                  # 8 output row-tiles of mm2 (D rows)
NCH = CAP // C               # chunks
S1, S2 = 4, 5                # power-of-2 prescales on W1', W2 (fp8 range fit)

BF16 = mybir.dt.bfloat16
FP8 = mybir.dt.float8e4
F32 = mybir.dt.float32
DR = mybir.MatmulPerfMode.DoubleRow
npbf16 = ml_dtypes.bfloat16
npfp8 = ml_dtypes.float8_e4m3

_nc_cache = {}


def _build(flags):
    """Build the per-core SPMD bass program. flags = (b1_nz, ns_nb_nz, b2_nz).

    The LayerNorm mean over H is folded into the weights on the host
    (W1' = (W1 - rowmean_H(W1)) * 2^S1, b1' = (b1 - mean(b1)) * 2^S1), so
    PSUM holds 2^S1 (h - mu) directly after the W1' matmul."""
    b1_nz, ns_nb_nz, b2_nz = flags
    nc = bacc.Bacc("TRN2", target_bir_lowering=False)

    xT_d = nc.dram_tensor("xT", [D, CAP], FP8, kind="ExternalInput")
    # chunk 0 of x duplicated in [P, KD*C] per-partition-contiguous layout:
    # loads in 2 partition-split DMAs (64 descriptors each) instead of 8
    # descriptor-serial ones, so the first matmul group starts ~4us earlier
    x0_d = nc.dram_tensor("x0", [P, KD * C], FP8, kind="ExternalInput")
    w1_d = nc.dram_tensor("w1", [D, H], FP8, kind="ExternalInput")
    w2_d = nc.dram_tensor("w2", [H, D], FP8, kind="ExternalInput")
    yT_d = nc.dram_tensor("yT", [D, CAP], F32, kind="ExternalOutput")
    if not ns_nb_nz:
        # fast path: rstd is a per-slot scalar; ship std to the host, which
        # folds 1/std (and b2) into the combine weights
        std_d = nc.dram_tensor("stdr", [1, CAP], F32, kind="ExternalOutput")
    if b1_nz:
        b1_d = nc.dram_tensor("b1", [H, 1], BF16, kind="ExternalInput")
    if ns_nb_nz:
        nsc_d = nc.dram_tensor("nsc", [H, 1], F32, kind="ExternalInput")
        nbs_d = nc.dram_tensor("nbs", [H, 1], F32, kind="ExternalInput")
    if b2_nz and ns_nb_nz:
        b2_d = nc.dram_tensor("b2", [D, 1], F32, kind="ExternalInput")

    xT_r = xT_d.rearrange("(ko p) c -> p ko c", p=P)
    x0_r = x0_d.rearrange("p (ko c) -> p ko c", ko=KD)
    w1_r = w1_d.rearrange("(ko p) h -> p ko h", p=P)
    w2_r = w2_d.rearrange("(ko p) d -> p ko d", p=P)
    yT_r = yT_d.rearrange("(dt p) c -> p dt c", p=P)

    with tile.TileContext(nc) as tc:
        with (
            tc.tile_pool(name="const", bufs=1) as cpool,
            tc.tile_pool(name="xp", bufs=3) as xpool,
            tc.tile_pool(name="hnp", bufs=2) as hnpool,
            tc.tile_pool(name="sqp", bufs=4) as sqpool,
            tc.tile_pool(name="rows", bufs=3) as rowpool,
            tc.tile_pool(name="rbp", bufs=2) as rbpool,
            tc.tile_pool(name="yp", bufs=3) as ypool,
            tc.tile_pool(name="hgen", bufs=2) as hgenpool,
            tc.tile_pool(name="ps_h", bufs=3, space="PSUM") as ps_h,
            tc.tile_pool(name="ps_y", bufs=4, space="PSUM") as ps_y,
            tc.tile_pool(name="ps_s", bufs=1, space="PSUM") as ps_s,
        ):
            # ---- resident constants (x chunk 0 first, then W1 in row-tile
            # slices so the first matmul group can start after ~0.7MB of DMA,
            # W2 last: not needed until the first mm2) ----------------------
            x_tiles = [None] * NCH

            def emit_x_load(c, split=False):
                x_tiles[c] = xpool.tile([P, KD, C], FP8, tag="x", name="x")
                if split:
                    for p0 in (0, 64):
                        nc.sync.dma_start(x_tiles[c][p0:p0 + 64, :, :],
                                          x0_r[p0:p0 + 64, :, :])
                else:
                    nc.sync.dma_start(x_tiles[c][:], xT_r[:, :, c * C:(c + 1) * C])

            # interleave the first x chunk with the first W1 row-tile slices
            # so the first matmul group's inputs land as early as possible
            w1_sb = cpool.tile([P, KD, H], FP8, tag="w1", name="w1")
            nc.sync.dma_start(w1_sb[:, :, 0:P], w1_r[:, :, 0:P])
            emit_x_load(0, split=True)
            # warm the PE HAM clock gate with dummy matmuls while the first
            # x/W1 DMAs land, so the first real matmuls run at 2.4 GHz
            warm = cpool.tile([P, 64], BF16, tag="warm", name="warm")
            nc.vector.memset(warm[:], 0.0)
            pwarm = ps_s.tile([P, C], F32, tag="small", name="small")
            for _ in range(32):
                nc.tensor.matmul(pwarm[:64, :64], lhsT=warm[:], rhs=warm[:],
                                 start=True, stop=True, skip_group_check=True)
            for mt in range(1, MT):
                nc.sync.dma_start(
                    w1_sb[:, :, mt * P:(mt + 1) * P], w1_r[:, :, mt * P:(mt + 1) * P]
                )
            ones_kcol = cpool.tile([P, 1], BF16, tag="ones_kcol", name="ones_kcol")
            nc.vector.memset(ones_kcol[:], 1.0)
            eps_sb = cpool.tile([1, 1], F32, tag="eps", name="eps")
            # Sqrt bias absorbs the power-of-2 prescales (see module docstring)
            if ns_nb_nz:
                nc.vector.memset(eps_sb[:], EPS * 2.0 ** (2 * S1))
            else:
                nc.vector.memset(eps_sb[:], EPS * 2.0 ** (2 * (S1 + S2)))
            if b1_nz:
                b1_sb = cpool.tile([1, H], BF16, tag="b1", name="b1")
                nc.sync.dma_start(b1_sb[:], b1_d.rearrange("h x -> x h"))
                ones_row = cpool.tile([1, C], BF16, tag="ones_row", name="ones_row")
                nc.vector.memset(ones_row[:], 1.0)
            if ns_nb_nz:
                nsc_sb = cpool.tile([P, MT], F32, tag="nsc", name="nsc")
                nc.sync.dma_start(nsc_sb[:], nsc_d.rearrange("(mt p) x -> p mt x", p=P)[:, :, 0])
                nbs_sb = cpool.tile([P, MT], F32, tag="nbs", name="nbs")
                nc.sync.dma_start(nbs_sb[:], nbs_d.rearrange("(mt p) x -> p mt x", p=P)[:, :, 0])
            if b2_nz and ns_nb_nz:
                b2_sb = cpool.tile([P, DT], F32, tag="b2", name="b2")
                nc.sync.dma_start(b2_sb[:], b2_d.rearrange("(dt p) x -> p dt x", p=P)[:, :, 0])
            w2_sb = cpool.tile([P, KH, D], FP8, tag="w2", name="w2")
            for kt in range(KH):
                nc.sync.dma_start(w2_sb[:, kt, :], w2_r[:, kt, :])

            for c in range(NCH):
                xt = x_tiles[c]
                hn = hnpool.tile([P, KH, C], FP8, tag="hn", name="hn")
                hflat = hgenpool.tile([P, KH, C], F32, tag="hflat", name="hflat") if ns_nb_nz else None
                # mm1: 16 row-tile groups of 4 DoubleRow matmuls (256
                # contraction each). ACT squares each PSUM tile (bf16), DVE
                # folds the squared tiles with a linear running sum (one add
                # per group window; after the LAST square only one add
                # remains, so the stats chain launches ~1us after the last
                # mm1 matmul), then PE does a single ones-matmul reduction.
                acc = None
                for mt in range(MT):
                    ph = ps_h.tile([P, C], F32, tag="ph", name="ph")
                    for kt in range(0, KD, 2):
                        nc.tensor.matmul(
                            ph[:], lhsT=w1_sb[:, kt:kt + 2, mt * P:(mt + 1) * P],
                            rhs=xt[:, kt:kt + 2, :], start=(kt == 0),
                            stop=(kt == KD - 2 and not b1_nz), perf_mode=DR,
                        )
                    if b1_nz:
                        nc.tensor.matmul(
                            ph[:], lhsT=b1_sb[:, mt * P:(mt + 1) * P], rhs=ones_row[:],
                            start=False, stop=True, skip_group_check=True,
                        )
                    sq = sqpool.tile([P, C], BF16, tag="sq", name="sq")
                    nc.scalar.square(sq[:], ph[:])
                    if ns_nb_nz:
                        nc.vector.tensor_copy(hflat[:, mt, :], ph[:])
                    else:
                        nc.vector.tensor_scalar_max(hn[:, mt, :], ph[:], 0.0)
                    if acc is None:
                        acc = sq
                    else:
                        nxt = sqpool.tile([P, C], BF16, tag=f"acc{mt % 2}",
                                          name="acc")
                        nc.vector.tensor_add(nxt[:], acc[:], sq[:])
                        acc = nxt
                hacc_bf = acc

                if c + 1 < NCH:
                    emit_x_load(c + 1)

                def emit_stats_head(ss, sqrt_scale):
                    # ss[1, C] = sum_p hacc_bf -> std -> rstd (row ops). The
                    # [1, C] DVE reciprocal costs ~3.2us; the whole chain is
                    # issued before the mm2 groups so 4 PSUM y-banks of PE
                    # run-ahead absorb it.
                    nc.tensor.matmul(ss[:1, :], lhsT=ones_kcol[:], rhs=hacc_bf[:],
                                     start=True, stop=True, skip_group_check=True)
                    std = rowpool.tile([1, C], F32, tag="std", name="std")
                    nc.scalar.activation(
                        std[:], ss[:1, :], mybir.ActivationFunctionType.Sqrt,
                        bias=eps_sb[:], scale=sqrt_scale,
                    )
                    rstd = rowpool.tile([1, C], F32, tag="rstd", name="rstd")
                    nc.vector.reciprocal(rstd[:], std[:])
                    return rstd

                def emit_rb(rstd):
                    # broadcast rstd across partitions on the (idle) GpSimd
                    rb = rbpool.tile([P, C], F32, tag="rb", name="rb")
                    nc.gpsimd.partition_broadcast(rb[:], rstd[:], channels=P)
                    return rb

                if ns_nb_nz:
                    # general path: hn = relu(((h-mu)*rstd)*nsc + nbs);
                    # rb here = rstd 2^-S1, and the 2^-S2 is applied at the
                    # mm2 eviction below.
                    ss = ps_s.tile([P, C], F32, tag="small", name="small")
                    rstd = emit_stats_head(ss, 1.0 / H)
                    rb = emit_rb(rstd)
                    for mt in range(MT):
                        tmp = hgenpool.tile([P, C], F32, tag="tmpn", name="tmpn")
                        nc.vector.tensor_mul(tmp[:], hflat[:, mt, :], rb[:])
                        nc.scalar.activation(
                            hn[:, mt, :], tmp[:],
                            mybir.ActivationFunctionType.Relu,
                            bias=nbs_sb[:, mt, None], scale=nsc_sb[:, mt, None],
                        )

                    for dt in range(DT):
                        py = ps_y.tile([P, C], F32, tag="py", name="py")
                        for kt in range(0, KH, 2):
                            nc.tensor.matmul(
                                py[:], lhsT=w2_sb[:, kt:kt + 2, dt * P:(dt + 1) * P],
                                rhs=hn[:, kt:kt + 2, :], start=(kt == 0),
                                stop=(kt == KH - 2), perf_mode=DR,
                            )
                        ysb = ypool.tile([P, C], F32, tag="y", name="y")
                        nc.vector.tensor_scalar_mul(ysb[:], py[:], 2.0 ** (-S2))
                        if b2_nz:
                            nc.vector.tensor_scalar_add(ysb[:], ysb[:], b2_sb[:, dt, None])
                        nc.sync.dma_start(yT_r[:, dt, c * C:(c + 1) * C], ysb[:])
                else:
                    # fast path: the per-slot rstd (and b2) are applied on
                    # the HOST during combine, so mm2 evictions are plain ACT
                    # copies with no dependency on the stats chain. The
                    # device only ships std = sqrt(ss*2^(2S2)/H + eps') rows.
                    pys = [None] * DT

                    def y_mms(dt):
                        pys[dt] = ps_y.tile([P, C], F32, tag="py", name="py")
                        for kt in range(0, KH, 2):
                            nc.tensor.matmul(
                                pys[dt][:], lhsT=w2_sb[:, kt:kt + 2, dt * P:(dt + 1) * P],
                                rhs=hn[:, kt:kt + 2, :], start=(kt == 0),
                                stop=(kt == KH - 2), perf_mode=DR,
                            )

                    def y_evict(dt):
                        ysb = ypool.tile([P, C], F32, tag="y", name="y")
                        nc.vector.tensor_copy(ysb[:], pys[dt][:])
                        if c == NCH - 1 and dt >= DT - 3:
                            # final stores are tail-exposed: halve their
                            # descriptor-serial time and issue from the (now
                            # idle) scalar queue
                            for p0 in (0, 64):
                                nc.scalar.dma_start(
                                    yT_r[p0:p0 + 64, dt, c * C:(c + 1) * C],
                                    ysb[p0:p0 + 64, :])
                        else:
                            nc.sync.dma_start(yT_r[:, dt, c * C:(c + 1) * C], ysb[:])

                    # stats ones-matmul sits in the PE's in-order stream:
                    # issue it after the first mm2 group so the PE never
                    # waits on the square/add chain draining
                    y_mms(0)
                    ss = ps_s.tile([P, C], F32, tag="small", name="small")
                    nc.tensor.matmul(ss[:1, :], lhsT=ones_kcol[:], rhs=hacc_bf[:],
                                     start=True, stop=True, skip_group_check=True)
                    std = rowpool.tile([1, C], F32, tag="std", name="std")
                    nc.scalar.activation(
                        std[:], ss[:1, :], mybir.ActivationFunctionType.Sqrt,
                        bias=eps_sb[:], scale=2.0 ** (2 * S2) / H,
                    )
                    nc.sync.dma_start(std_d[:, c * C:(c + 1) * C], std[:])
                    y_evict(0)
                    for dt in range(1, DT):
                        y_mms(dt)
                        y_evict(dt)

    nc.compile()
    return nc


# ------------------------------------------------------------ host logic ---
def _route(x0, ln_scale, ln_bias, Wr, br):
    """LayerNorm -> relu -> router logits -> top-2 -> gates (float64 math)."""
    x = x0.astype(np.float64)
    mu = x.mean(axis=-1, keepdims=True)
    var = np.square(x - mu).mean(axis=-1, keepdims=True)
    xn = (x - mu) / np.sqrt(var + EPS)
    xn = xn * ln_scale.astype(np.float64) + ln_bias.astype(np.float64)
    np.maximum(xn, 0.0, out=xn)
    logits = xn @ Wr.astype(np.float64) + br.astype(np.float64)

    n = logits.shape[0]
    rows = np.arange(n)
    i0 = np.argmax(logits, axis=1)
    l0 = logits[rows, i0]
    tmp = logits.copy()
    tmp[rows, i0] = -np.inf
    i1 = np.argmax(tmp, axis=1)
    l1 = tmp[rows, i1]
    # softmax over (l0, l1); l0 >= l1
    e1 = np.exp(l1 - l0)
    g0 = 1.0 / (1.0 + e1)
    g1 = e1 / (1.0 + e1)
    top_idx = np.stack([i0, i1], axis=1).astype(np.int64)
    gates = np.stack([g0, g1], axis=1)
    return xn.astype(np.float32), top_idx, gates


def _positions(top_idx):
    """Capacity positions: running per-expert count in token-major slot order."""
    eidx = top_idx.reshape(-1)
    nk = eidx.shape[0]
    oh = (eidx[:, None] == np.arange(E)[None, :]).astype(np.int64)
    pos = np.cumsum(oh, axis=0)[np.arange(nk), eidx] - 1
    mask = pos < CAP
    pos_c = np.minimum(pos, CAP - 1)
    return eidx, pos, pos_c, mask


def kernel(**inputs):
    x0 = np.asarray(inputs["x0"], np.float32)
    ln_scale = np.asarray(inputs["ln_scale"], np.float32)
    ln_bias = np.asarray(inputs["ln_bias"], np.float32)
    Wr = np.asarray(inputs["Wr"], np.float32)
    br = np.asarray(inputs["br"], np.float32)
    W1 = np.asarray(inputs["W1"], np.float32)
    b1 = np.asarray(inputs["b1"], np.float32)
    n_scale = np.asarray(inputs["n_scale"], np.float32)
    n_bias = np.asarray(inputs["n_bias"], np.float32)
    W2 = np.asarray(inputs["W2"], np.float32)
    b2 = np.asarray(inputs["b2"], np.float32)

    # ---- host routing + dispatch ---------------------------------------
    xn, top_idx, gates = _route(x0, ln_scale, ln_bias, Wr, br)
    eidx, pos, pos_c, mask = _positions(top_idx)

    tok_of_slot = np.repeat(np.arange(N), TOPK)
    keep = mask
    expert_inputs = np.zeros((E, CAP, D), np.float32)
    expert_inputs[eidx[keep], pos[keep]] = xn[tok_of_slot[keep]]

    # ---- build / fetch compiled program --------------------------------
    b1_nz = bool(np.any(b1))
    ns_nb_nz = bool(np.any(n_scale != 1.0) or np.any(n_bias))
    b2_nz = bool(np.any(b2))
    flags = (b1_nz, ns_nb_nz, b2_nz)
    if flags not in _nc_cache:
        _nc_cache[flags] = _build(flags)
    nc = _nc_cache[flags]

    # ---- per-core inputs ----------------------------------------------
    in_maps = []
    for e in range(E):
        # Fold the LayerNorm mean over H into the weights: x @ W1' = h - mu.
        w1p = W1[e].astype(np.float64)
        w1p = w1p - w1p.mean(axis=1, keepdims=True)
        xT8 = np.ascontiguousarray(expert_inputs[e].T).astype(npfp8)
        m = {
            "xT": xT8,
            # chunk 0 duplicated in [P, KD*C] per-partition-contiguous layout
            "x0": np.ascontiguousarray(
                xT8[:, :C].reshape(KD, P, C).transpose(1, 0, 2).reshape(P, KD * C)),
            "w1": (w1p * 2.0 ** S1).astype(npfp8),
            "w2": (W2[e].astype(np.float64) * 2.0 ** S2).astype(npfp8),
        }
        if b1_nz:
            b1p = b1[e].astype(np.float64)
            b1p = b1p - b1p.mean()
            m["b1"] = (b1p * 2.0 ** S1).astype(npbf16)[:, None]
        if ns_nb_nz:
            m["nsc"] = n_scale[e].astype(np.float32)[:, None]
            m["nbs"] = n_bias[e].astype(np.float32)[:, None]
        if b2_nz and ns_nb_nz:
            m["b2"] = b2[e].astype(np.float32)[:, None]
        in_maps.append(m)

    res = run_bass_kernel_spmd(nc, in_maps, core_ids=list(range(E)))

    # ---- combine -------------------------------------------------------
    yT_all = np.stack([res.results[e]["yT"] for e in range(E)])  # [E, D, CAP]
    w = (gates.astype(np.float32) * mask.reshape(N, TOPK))
    pos2 = pos_c.reshape(N, TOPK)
    mix = np.zeros((N, D), np.float32)
    if ns_nb_nz:
        for k in range(TOPK):
            mix += yT_all[top_idx[:, k], :, pos2[:, k]] * w[:, k:k + 1]
    else:
        # fast path: device output is raw 2^(S1+S2) W2^T relu(h-mu); fold
        # rstd 2^-(S1+S2) = 1/std (and b2) into the per-slot combine weights
        std_all = np.stack([np.asarray(res.results[e]["stdr"], np.float64)
                            .reshape(CAP) for e in range(E)])  # [E, CAP]
        rb_all = 1.0 / std_all
        for k in range(TOPK):
            wk = (w[:, k] * rb_all[top_idx[:, k], pos2[:, k]]).astype(np.float32)
            mix += yT_all[top_idx[:, k], :, pos2[:, k]] * wk[:, None]
            if b2_nz:
                mix += b2[top_idx[:, k]] * w[:, k:k + 1]
    return x0 + mix


# revision 28
# speedup vs baseline: 1.2558x; 1.0002x over previous
"""MoE pre-activation residual block on 8 trn2 NeuronCores (expert-parallel).

kernel(**inputs) takes the full unsharded inputs (numpy, keyed as in
setup_inputs) and returns the full [N, D] float32 output.

Host: LayerNorm+relu, router logits, top-2 gating, capacity-based dispatch
      (builds expert_inputs per expert), final gather/combine/residual.
Device (one expert per core, SPMD): the expert MLP
      y = LN_h(x @ W1 + b1) -> relu -> @ W2 + b2
  computed as two fp8(e4m3) DoubleRow matmuls with fp32 PSUM accumulation
  (each matmul instruction contracts 256 = 2x128 via paired k-subtiles):
    - h^T[H, c] = sum_d W1'[d, h] x^T[d, c]  (lhsT = W1 as stored); the LN
      mean over H is folded into the weights on the host
      (W1' = (W1 - rowmean_H(W1)) * 2^S1), so PSUM holds 2^S1 (h - mu)
    - var = sum_H (h-mu)^2: ACT squares each PSUM tile (bf16), DVE folds the
      16 tiles with a pairwise add tree, PE does ONE ones-matmul reduction;
      the 2^S1 / 2^S2 prescales fold exactly into the Sqrt bias/scale
    - hn = relu(2^S1 (h - mu)) cast to fp8 straight from PSUM (rstd > 0
      commutes w/ relu, so it can be applied after mm2)
    - y^T[D, c] = sum_h (2^S2 W2[h, d]) hn[h, c]; rb = rstd 2^-(S1+S2)
      (broadcast across partitions by GpSimd) applied at PSUM eviction on DVE
"""

import sys

try:
    import concourse.bacc  # noqa: F401
except ImportError:  # pragma: no cover
    for _p in ("/opt/trn_rl_repo", "/root/.axon_site/_ro/trn_rl_repo"):
        if _p not in sys.path:
            sys.path.append(_p)

import numpy as np
import ml_dtypes

import concourse.bacc as bacc
import concourse.mybir as mybir
import concourse.tile as tile
from concourse.bass_utils import run_bass_kernel_spmd

# ---------------------------------------------------------------- shim -----
# Under axon, run_bass_kernel_spmd(trace=True) needs antenv.axon_hooks for
# NTFF profiling. Some images lack it; register an equivalent hook so a
# BASS_TRACE=1 run still produces timing instead of silently skipping.
def _install_axon_hooks_shim():
    try:
        import antenv.axon_hooks  # noqa: F401
        return
    except ImportError:
        pass
    import contextlib, ctypes, types, os

    so = "/opt/axon/libaxon_pjrt.so"
    hook = None
    if os.path.exists(so):
        try:
            lib = ctypes.CDLL(so)
            if hasattr(lib, "axon_start_nrt_profile"):
                lib.axon_start_nrt_profile.argtypes = [
                    ctypes.POINTER(ctypes.c_int64),
                    ctypes.c_size_t,
                ]
                lib.axon_start_nrt_profile.restype = ctypes.c_int64
                lib.axon_stop_nrt_profile.argtypes = [ctypes.c_char_p]
                lib.axon_stop_nrt_profile.restype = ctypes.c_int64

                @contextlib.contextmanager
                def _hook(output_dir, device_ids):
                    import jax

                    jax.devices()
                    if device_ids:
                        ids = (ctypes.c_int64 * len(device_ids))(*device_ids)
                        rc = lib.axon_start_nrt_profile(ids, len(device_ids))
                    else:
                        rc = lib.axon_start_nrt_profile(None, 0)
                    if rc != 0:
                        raise RuntimeError(f"axon_start_nrt_profile rc={rc}")
                    try:
                        yield
                    finally:
                        n = lib.axon_stop_nrt_profile(str(output_dir).encode())
                        print(f"ntff profile: {n} file(s) -> {output_dir}",
                              file=sys.stderr)

                hook = _hook
        except OSError:
            hook = None
    mod = types.ModuleType("antenv.axon_hooks")
    mod.get_axon_ntff_profile_hook = lambda: hook
    mod.set_axon_ntff_profile_hook = lambda h: None
    sys.modules["antenv.axon_hooks"] = mod


_install_axon_hooks_shim()

# ------------------------------------------------------------- constants ---
N, D, H, E, TOPK = 16384, 1024, 2048, 8, 2
CAP = 4096
EPS = 1e-6
P = 128
C = 512                      # CAP-chunk (columns per pipeline step)
KD, KH = D // P, H // P      # 8 k-subtiles for mm1, 16 for mm2
MT = H // P                  # 16 output row-tiles of mm1 (H rows)
DT = D // P                  # 8 output row-tiles of mm2 (D rows)
NCH = CAP // C               # chunks
S1, S2 = 4, 5                # power-of-2 prescales on W1', W2 (fp8 range fit)

BF16 = mybir.dt.bfloat16
FP8 = mybir.dt.float8e4
F32 = mybir.dt.float32
DR = mybir.MatmulPerfMode.DoubleRow
npbf16 = ml_dtypes.bfloat16
npfp8 = ml_dtypes.float8_e4m3

_nc_cache = {}


def _build(flags):
    """Build the per-core SPMD bass program. flags = (b1_nz, ns_nb_nz, b2_nz).

    The LayerNorm mean over H is folded into the weights on the host
    (W1' = (W1 - rowmean_H(W1)) * 2^S1, b1' = (b1 - mean(b1)) * 2^S1), so
    PSUM holds 2^S1 (h - mu) directly after the W1' matmul."""
    b1_nz, ns_nb_nz, b2_nz = flags
    nc = bacc.Bacc("TRN2", target_bir_lowering=False)

    # x and W1 come in host-relaid per-partition-contiguous layouts: every
    # DMA descriptor covers a 1-4KB contiguous line instead of 128-512B
    # segments (the [D, CAP]/[D, H] layouts cost ~24k descriptors per core,
    # saturating the DMA engines' ~32ns/descriptor processing rate)
    xT_d = nc.dram_tensor("xT", [P, NCH * KD * C], FP8, kind="ExternalInput")
    w1_d = nc.dram_tensor("w1", [P, MT * KD * P], FP8, kind="ExternalInput")
    w2_d = nc.dram_tensor("w2", [H, D], FP8, kind="ExternalInput")
    yT_d = nc.dram_tensor("yT", [D, CAP], F32, kind="ExternalOutput")
    if not ns_nb_nz:
        # fast path: rstd is a per-slot scalar; ship std to the host, which
        # folds 1/std (and b2) into the combine weights
        std_d = nc.dram_tensor("stdr", [1, CAP], F32, kind="ExternalOutput")
    if b1_nz:
        b1_d = nc.dram_tensor("b1", [H, 1], BF16, kind="ExternalInput")
    if ns_nb_nz:
        nsc_d = nc.dram_tensor("nsc", [H, 1], F32, kind="ExternalInput")
        nbs_d = nc.dram_tensor("nbs", [H, 1], F32, kind="ExternalInput")
    if b2_nz and ns_nb_nz:
        b2_d = nc.dram_tensor("b2", [D, 1], F32, kind="ExternalInput")

    xT_r = xT_d.rearrange("p (nch ko c) -> p nch ko c", nch=NCH, ko=KD)
    w1_r = w1_d.rearrange("p (mt ko j) -> p mt ko j", mt=MT, ko=KD)
    w2_r = w2_d.rearrange("(ko p) d -> p ko d", p=P)
    yT_r = yT_d.rearrange("(dt p) c -> p dt c", p=P)

    with tile.TileContext(nc) as tc:
        with (
            tc.tile_pool(name="const", bufs=1) as cpool,
            tc.tile_pool(name="xp", bufs=3) as xpool,
            tc.tile_pool(name="hnp", bufs=2) as hnpool,
            tc.tile_pool(name="sqp", bufs=4) as sqpool,
            tc.tile_pool(name="rows", bufs=3) as rowpool,
            tc.tile_pool(name="rbp", bufs=2) as rbpool,
            tc.tile_pool(name="yp", bufs=3) as ypool,
            tc.tile_pool(name="hgen", bufs=2) as hgenpool,
            tc.tile_pool(name="ps_h", bufs=3, space="PSUM") as ps_h,
            tc.tile_pool(name="ps_y", bufs=4, space="PSUM") as ps_y,
            tc.tile_pool(name="ps_s", bufs=1, space="PSUM") as ps_s,
        ):
            # ---- resident constants (x chunk 0 first, then W1 in row-tile
            # slices so the first matmul group can start after ~0.7MB of DMA,
            # W2 last: not needed until the first mm2) ----------------------
            x_tiles = [None] * NCH

            def emit_x_load(c, split=False):
                x_tiles[c] = xpool.tile([P, KD, C], FP8, tag="x", name="x")
                if split:
                    # chunk 0 gates the first matmul: 4 partition strips on
                    # two issue queues
                    for i, p0 in enumerate(range(0, P, 32)):
                        eng = nc.sync if i % 2 == 0 else nc.scalar
                        eng.dma_start(x_tiles[c][p0:p0 + 32, :, :],
                                      xT_r[p0:p0 + 32, c, :, :])
                else:
                    for p0 in (0, 64):
                        nc.sync.dma_start(x_tiles[c][p0:p0 + 64, :, :],
                                          xT_r[p0:p0 + 64, c, :, :])

            # interleave the first x chunk with the first W1 row-tile slices
            # so the first matmul group's inputs land as early as possible
            # (w1_sb is mt-major so each slice load is 1KB-contiguous)
            w1_sb = cpool.tile([P, MT, KD, P], FP8, tag="w1", name="w1")
            for p0 in (0, 64):
                nc.sync.dma_start(w1_sb[p0:p0 + 64, 0, :, :],
                                  w1_r[p0:p0 + 64, 0, :, :])
            emit_x_load(0, split=True)
            # warm the PE HAM clock gate with dummy matmuls while the first
            # x/W1 DMAs land, so the first real matmuls run at 2.4 GHz
            warm = cpool.tile([P, 64], BF16, tag="warm", name="warm")
            nc.vector.memset(warm[:], 0.0)
            pwarm = ps_s.tile([P, C], F32, tag="small", name="small")
            for _ in range(32):
                nc.tensor.matmul(pwarm[:64, :64], lhsT=warm[:], rhs=warm[:],
                                 start=True, stop=True, skip_group_check=True)
            for mt in range(1, MT):
                nc.sync.dma_start(w1_sb[:, mt, :, :], w1_r[:, mt, :, :])
            ones_kcol = cpool.tile([P, 1], BF16, tag="ones_kcol", name="ones_kcol")
            nc.vector.memset(ones_kcol[:], 1.0)
            eps_sb = cpool.tile([1, 1], F32, tag="eps", name="eps")
            # Sqrt bias absorbs the power-of-2 prescales (see module docstring)
            if ns_nb_nz:
                nc.vector.memset(eps_sb[:], EPS * 2.0 ** (2 * S1))
            else:
                nc.vector.memset(eps_sb[:], EPS * 2.0 ** (2 * (S1 + S2)))
            if b1_nz:
                b1_sb = cpool.tile([1, H], BF16, tag="b1", name="b1")
                nc.sync.dma_start(b1_sb[:], b1_d.rearrange("h x -> x h"))
                ones_row = cpool.tile([1, C], BF16, tag="ones_row", name="ones_row")
                nc.vector.memset(ones_row[:], 1.0)
            if ns_nb_nz:
                nsc_sb = cpool.tile([P, MT], F32, tag="nsc", name="nsc")
                nc.sync.dma_start(nsc_sb[:], nsc_d.rearrange("(mt p) x -> p mt x", p=P)[:, :, 0])
                nbs_sb = cpool.tile([P, MT], F32, tag="nbs", name="nbs")
                nc.sync.dma_start(nbs_sb[:], nbs_d.rearrange("(mt p) x -> p mt x", p=P)[:, :, 0])
            if b2_nz and ns_nb_nz:
                b2_sb = cpool.tile([P, DT], F32, tag="b2", name="b2")
                nc.sync.dma_start(b2_sb[:], b2_d.rearrange("(dt p) x -> p dt x", p=P)[:, :, 0])
            w2_sb = cpool.tile([P, KH, D], FP8, tag="w2", name="w2")
            for kt in range(KH):
                nc.sync.dma_start(w2_sb[:, kt, :], w2_r[:, kt, :])

            for c in range(NCH):
                xt = x_tiles[c]
                hn = hnpool.tile([P, KH, C], FP8, tag="hn", name="hn")
                hflat = hgenpool.tile([P, KH, C], F32, tag="hflat", name="hflat") if ns_nb_nz else None
                # mm1: 16 row-tile groups of 4 DoubleRow matmuls (256
                # contraction each). ACT squares each PSUM tile (bf16), DVE
                # folds the squared tiles with a linear running sum (one add
                # per group window; after the LAST square only one add
                # remains, so the stats chain launches ~1us after the last
                # mm1 matmul), then PE does a single ones-matmul reduction.
                acc = None
                for mt in range(MT):
                    ph = ps_h.tile([P, C], F32, tag="ph", name="ph")
                    for kt in range(0, KD, 2):
                        nc.tensor.matmul(
                            ph[:], lhsT=w1_sb[:, mt, kt:kt + 2, :],
                            rhs=xt[:, kt:kt + 2, :], start=(kt == 0),
                            stop=(kt == KD - 2 and not b1_nz), perf_mode=DR,
                        )
                    if b1_nz:
                        nc.tensor.matmul(
                            ph[:], lhsT=b1_sb[:, mt * P:(mt + 1) * P], rhs=ones_row[:],
                            start=False, stop=True, skip_group_check=True,
                        )
                    sq = sqpool.tile([P, C], BF16, tag="sq", name="sq")
                    nc.scalar.square(sq[:], ph[:])
                    if ns_nb_nz:
                        nc.vector.tensor_copy(hflat[:, mt, :], ph[:])
                    else:
                        nc.vector.tensor_scalar_max(hn[:, mt, :], ph[:], 0.0)
                    if acc is None:
                        acc = sq
                    else:
                        nxt = sqpool.tile([P, C], BF16, tag=f"acc{mt % 2}",
                                          name="acc")
                        nc.vector.tensor_add(nxt[:], acc[:], sq[:])
                        acc = nxt
                hacc_bf = acc

                if c + 1 < NCH:
                    emit_x_load(c + 1)

                def emit_stats_head(ss, sqrt_scale):
                    # ss[1, C] = sum_p hacc_bf -> std -> rstd (row ops). The
                    # [1, C] DVE reciprocal costs ~3.2us; the whole chain is
                    # issued before the mm2 groups so 4 PSUM y-banks of PE
                    # run-ahead absorb it.
                    nc.tensor.matmul(ss[:1, :], lhsT=ones_kcol[:], rhs=hacc_bf[:],
                                     start=True, stop=True, skip_group_check=True)
                    std = rowpool.tile([1, C], F32, tag="std", name="std")
                    nc.scalar.activation(
                        std[:], ss[:1, :], mybir.ActivationFunctionType.Sqrt,
                        bias=eps_sb[:], scale=sqrt_scale,
                    )
                    rstd = rowpool.tile([1, C], F32, tag="rstd", name="rstd")
                    nc.vector.reciprocal(rstd[:], std[:])
                    return rstd

                def emit_rb(rstd):
                    # broadcast rstd across partitions on the (idle) GpSimd
                    rb = rbpool.tile([P, C], F32, tag="rb", name="rb")
                    nc.gpsimd.partition_broadcast(rb[:], rstd[:], channels=P)
                    return rb

                if ns_nb_nz:
                    # general path: hn = relu(((h-mu)*rstd)*nsc + nbs);
                    # rb here = rstd 2^-S1, and the 2^-S2 is applied at the
                    # mm2 eviction below.
                    ss = ps_s.tile([P, C], F32, tag="small", name="small")
                    rstd = emit_stats_head(ss, 1.0 / H)
                    rb = emit_rb(rstd)
                    for mt in range(MT):
                        tmp = hgenpool.tile([P, C], F32, tag="tmpn", name="tmpn")
                        nc.vector.tensor_mul(tmp[:], hflat[:, mt, :], rb[:])
                        nc.scalar.activation(
                            hn[:, mt, :], tmp[:],
                            mybir.ActivationFunctionType.Relu,
                            bias=nbs_sb[:, mt, None], scale=nsc_sb[:, mt, None],
                        )

                    for dt in range(DT):
                        py = ps_y.tile([P, C], F32, tag="py", name="py")
                        for kt in range(0, KH, 2):
                            nc.tensor.matmul(
                                py[:], lhsT=w2_sb[:, kt:kt + 2, dt * P:(dt + 1) * P],
                                rhs=hn[:, kt:kt + 2, :], start=(kt == 0),
                                stop=(kt == KH - 2), perf_mode=DR,
                            )
                        ysb = ypool.tile([P, C], F32, tag="y", name="y")
                        nc.vector.tensor_scalar_mul(ysb[:], py[:], 2.0 ** (-S2))
                        if b2_nz:
                            nc.vector.tensor_scalar_add(ysb[:], ysb[:], b2_sb[:, dt, None])
                        nc.sync.dma_start(yT_r[:, dt, c * C:(c + 1) * C], ysb[:])
                else:
                    # fast path: the per-slot rstd (and b2) are applied on
                    # the HOST during combine, so mm2 evictions are plain ACT
                    # copies with no dependency on the stats chain. The
                    # device only ships std = sqrt(ss*2^(2S2)/H + eps') rows.
                    pys = [None] * DT

                    def y_mms(dt):
                        pys[dt] = ps_y.tile([P, C], F32, tag="py", name="py")
                        for kt in range(0, KH, 2):
                            nc.tensor.matmul(
                                pys[dt][:], lhsT=w2_sb[:, kt:kt + 2, dt * P:(dt + 1) * P],
                                rhs=hn[:, kt:kt + 2, :], start=(kt == 0),
                                stop=(kt == KH - 2), perf_mode=DR,
                            )

                    def y_evict(dt):
                        ysb = ypool.tile([P, C], F32, tag="y", name="y")
                        nc.vector.tensor_copy(ysb[:], pys[dt][:])
                        if c == NCH - 1 and dt >= DT - 3:
                            # final stores are tail-exposed: halve their
                            # descriptor-serial time and issue from the (now
                            # idle) scalar queue
                            for p0 in (0, 64):
                                nc.scalar.dma_start(
                                    yT_r[p0:p0 + 64, dt, c * C:(c + 1) * C],
                                    ysb[p0:p0 + 64, :])
                        else:
                            nc.sync.dma_start(yT_r[:, dt, c * C:(c + 1) * C], ysb[:])

                    # stats ones-matmul sits in the PE's in-order stream:
                    # issue it after the first mm2 group so the PE never
                    # waits on the square/add chain draining
                    y_mms(0)
                    ss = ps_s.tile([P, C], F32, tag="small", name="small")
                    nc.tensor.matmul(ss[:1, :], lhsT=ones_kcol[:], rhs=hacc_bf[:],
                                     start=True, stop=True, skip_group_check=True)
                    std = rowpool.tile([1, C], F32, tag="std", name="std")
                    nc.scalar.activation(
                        std[:], ss[:1, :], mybir.ActivationFunctionType.Sqrt,
                        bias=eps_sb[:], scale=2.0 ** (2 * S2) / H,
                    )
                    nc.sync.dma_start(std_d[:, c * C:(c + 1) * C], std[:])
                    y_evict(0)
                    for dt in range(1, DT):
                        y_mms(dt)
                        y_evict(dt)

    nc.compile()
    return nc


# ------------------------------------------------------------ host logic ---
def _route(x0, ln_scale, ln_bias, Wr, br):
    """LayerNorm -> relu -> router logits -> top-2 -> gates (float64 math)."""
    x = x0.astype(np.float64)
    mu = x.mean(axis=-1, keepdims=True)
    var = np.square(x - mu).mean(axis=-1, keepdims=True)
    xn = (x - mu) / np.sqrt(var + EPS)
    xn = xn * ln_scale.astype(np.float64) + ln_bias.astype(np.float64)
    np.maximum(xn, 0.0, out=xn)
    logits = xn @ Wr.astype(np.float64) + br.astype(np.float64)

    n = logits.shape[0]
    rows = np.arange(n)
    i0 = np.argmax(logits, axis=1)
    l0 = logits[rows, i0]
    tmp = logits.copy()
    tmp[rows, i0] = -np.inf
    i1 = np.argmax(tmp, axis=1)
    l1 = tmp[rows, i1]
    # softmax over (l0, l1); l0 >= l1
    e1 = np.exp(l1 - l0)
    g0 = 1.0 / (1.0 + e1)
    g1 = e1 / (1.0 + e1)
    top_idx = np.stack([i0, i1], axis=1).astype(np.int64)
    gates = np.stack([g0, g1], axis=1)
    return xn.astype(np.float32), top_idx, gates


def _positions(top_idx):
    """Capacity positions: running per-expert count in token-major slot order."""
    eidx = top_idx.reshape(-1)
    nk = eidx.shape[0]
    oh = (eidx[:, None] == np.arange(E)[None, :]).astype(np.int64)
    pos = np.cumsum(oh, axis=0)[np.arange(nk), eidx] - 1
    mask = pos < CAP
    pos_c = np.minimum(pos, CAP - 1)
    return eidx, pos, pos_c, mask


def kernel(**inputs):
    x0 = np.asarray(inputs["x0"], np.float32)
    ln_scale = np.asarray(inputs["ln_scale"], np.float32)
    ln_bias = np.asarray(inputs["ln_bias"], np.float32)
    Wr = np.asarray(inputs["Wr"], np.float32)
    br = np.asarray(inputs["br"], np.float32)
    W1 = np.asarray(inputs["W1"], np.float32)
    b1 = np.asarray(inputs["b1"], np.float32)
    n_scale = np.asarray(inputs["n_scale"], np.float32)
    n_bias = np.asarray(inputs["n_bias"], np.float32)
    W2 = np.asarray(inputs["W2"], np.float32)
    b2 = np.asarray(inputs["b2"], np.float32)

    # ---- host routing + dispatch ---------------------------------------
    xn, top_idx, gates = _route(x0, ln_scale, ln_bias, Wr, br)
    eidx, pos, pos_c, mask = _positions(top_idx)

    tok_of_slot = np.repeat(np.arange(N), TOPK)
    keep = mask
    expert_inputs = np.zeros((E, CAP, D), np.float32)
    expert_inputs[eidx[keep], pos[keep]] = xn[tok_of_slot[keep]]

    # ---- build / fetch compiled program --------------------------------
    b1_nz = bool(np.any(b1))
    ns_nb_nz = bool(np.any(n_scale != 1.0) or np.any(n_bias))
    b2_nz = bool(np.any(b2))
    flags = (b1_nz, ns_nb_nz, b2_nz)
    if flags not in _nc_cache:
        _nc_cache[flags] = _build(flags)
    nc = _nc_cache[flags]

    # ---- per-core inputs ----------------------------------------------
    in_maps = []
    for e in range(E):
        # Fold the LayerNorm mean over H into the weights: x @ W1' = h - mu.
        w1p = W1[e].astype(np.float64)
        w1p = w1p - w1p.mean(axis=1, keepdims=True)
        # per-partition-contiguous relayouts (see _build): x [P, NCH*KD*C],
        # W1 [P, MT*KD*128]
        xT8 = expert_inputs[e].T.astype(npfp8)            # [D, CAP]
        x_lay = np.ascontiguousarray(
            xT8.reshape(KD, P, NCH, C).transpose(1, 2, 0, 3).reshape(P, -1))
        w1q = (w1p * 2.0 ** S1).astype(npfp8)             # [D, H]
        w1_lay = np.ascontiguousarray(
            w1q.reshape(KD, P, MT, P).transpose(1, 2, 0, 3).reshape(P, -1))
        m = {
            "xT": x_lay,
            "w1": w1_lay,
            "w2": (W2[e].astype(np.float64) * 2.0 ** S2).astype(npfp8),
        }
        if b1_nz:
            b1p = b1[e].astype(np.float64)
            b1p = b1p - b1p.mean()
            m["b1"] = (b1p * 2.0 ** S1).astype(npbf16)[:, None]
        if ns_nb_nz:
            m["nsc"] = n_scale[e].astype(np.float32)[:, None]
            m["nbs"] = n_bias[e].astype(np.float32)[:, None]
        if b2_nz and ns_nb_nz:
            m["b2"] = b2[e].astype(np.float32)[:, None]
        in_maps.append(m)

    res = run_bass_kernel_spmd(nc, in_maps, core_ids=list(range(E)))

    # ---- combine -------------------------------------------------------
    yT_all = np.stack([res.results[e]["yT"] for e in range(E)])  # [E, D, CAP]
    w = (gates.astype(np.float32) * mask.reshape(N, TOPK))
    pos2 = pos_c.reshape(N, TOPK)
    mix = np.zeros((N, D), np.float32)
    if ns_nb_nz:
        for k in range(TOPK):
            mix += yT_all[top_idx[:, k], :, pos2[:, k]] * w[:, k:k + 1]
    else:
        # fast path: device output is raw 2^(S1+S2) W2^T relu(h-mu); fold
        # rstd 2^-(S1+S2) = 1/std (and b2) into the per-slot combine weights
        std_all = np.stack([np.asarray(res.results[e]["stdr"], np.float64)
                            .reshape(CAP) for e in range(E)])  # [E, CAP]
        rb_all = 1.0 / std_all
        for k in range(TOPK):
            wk = (w[:, k] * rb_all[top_idx[:, k], pos2[:, k]]).astype(np.float32)
            mix += yT_all[top_idx[:, k], :, pos2[:, k]] * wk[:, None]
            if b2_nz:
                mix += b2[top_idx[:, k]] * w[:, k:k + 1]
    return x0 + mix


# revision 30
# speedup vs baseline: 1.2796x; 1.0189x over previous
"""MoE pre-activation residual block on 8 trn2 NeuronCores (expert-parallel).

kernel(**inputs) takes the full unsharded inputs (numpy, keyed as in
setup_inputs) and returns the full [N, D] float32 output.

Host: LayerNorm+relu, router logits, top-2 gating, capacity-based dispatch
      (builds expert_inputs per expert), final gather/combine/residual.
Device (one expert per core, SPMD): the expert MLP
      y = LN_h(x @ W1 + b1) -> relu -> @ W2 + b2
  computed as two fp8(e4m3) DoubleRow matmuls with fp32 PSUM accumulation
  (each matmul instruction contracts 256 = 2x128 via paired k-subtiles):
    - h^T[H, c] = sum_d W1'[d, h] x^T[d, c]  (lhsT = W1 as stored); the LN
      mean over H is folded into the weights on the host
      (W1' = (W1 - rowmean_H(W1)) * 2^S1), so PSUM holds 2^S1 (h - mu)
    - var = sum_H (h-mu)^2: ACT squares each PSUM tile (bf16), DVE folds the
      16 tiles with a pairwise add tree, PE does ONE ones-matmul reduction;
      the 2^S1 / 2^S2 prescales fold exactly into the Sqrt bias/scale
    - hn = relu(2^S1 (h - mu)) cast to fp8 straight from PSUM (rstd > 0
      commutes w/ relu, so it can be applied after mm2)
    - y^T[D, c] = sum_h (2^S2 W2[h, d]) hn[h, c]; rb = rstd 2^-(S1+S2)
      (broadcast across partitions by GpSimd) applied at PSUM eviction on DVE
"""

import sys

try:
    import concourse.bacc  # noqa: F401
except ImportError:  # pragma: no cover
    for _p in ("/opt/trn_rl_repo", "/root/.axon_site/_ro/trn_rl_repo"):
        if _p not in sys.path:
            sys.path.append(_p)

import numpy as np
import ml_dtypes

import concourse.bacc as bacc
import concourse.mybir as mybir
import concourse.tile as tile
from concourse.bass_utils import run_bass_kernel_spmd

# ---------------------------------------------------------------- shim -----
# Under axon, run_bass_kernel_spmd(trace=True) needs antenv.axon_hooks for
# NTFF profiling. Some images lack it; register an equivalent hook so a
# BASS_TRACE=1 run still produces timing instead of silently skipping.
def _install_axon_hooks_shim():
    try:
        import antenv.axon_hooks  # noqa: F401
        return
    except ImportError:
        pass
    import contextlib, ctypes, types, os

    so = "/opt/axon/libaxon_pjrt.so"
    hook = None
    if os.path.exists(so):
        try:
            lib = ctypes.CDLL(so)
            if hasattr(lib, "axon_start_nrt_profile"):
                lib.axon_start_nrt_profile.argtypes = [
                    ctypes.POINTER(ctypes.c_int64),
                    ctypes.c_size_t,
                ]
                lib.axon_start_nrt_profile.restype = ctypes.c_int64
                lib.axon_stop_nrt_profile.argtypes = [ctypes.c_char_p]
                lib.axon_stop_nrt_profile.restype = ctypes.c_int64

                @contextlib.contextmanager
                def _hook(output_dir, device_ids):
                    import jax

                    jax.devices()
                    if device_ids:
                        ids = (ctypes.c_int64 * len(device_ids))(*device_ids)
                        rc = lib.axon_start_nrt_profile(ids, len(device_ids))
                    else:
                        rc = lib.axon_start_nrt_profile(None, 0)
                    if rc != 0:
                        raise RuntimeError(f"axon_start_nrt_profile rc={rc}")
                    try:
                        yield
                    finally:
                        n = lib.axon_stop_nrt_profile(str(output_dir).encode())
                        print(f"ntff profile: {n} file(s) -> {output_dir}",
                              file=sys.stderr)

                hook = _hook
        except OSError:
            hook = None
    mod = types.ModuleType("antenv.axon_hooks")
    mod.get_axon_ntff_profile_hook = lambda: hook
    mod.set_axon_ntff_profile_hook = lambda h: None
    sys.modules["antenv.axon_hooks"] = mod


_install_axon_hooks_shim()

# ------------------------------------------------------------- constants ---
N, D, H, E, TOPK = 16384, 1024, 2048, 8, 2
CAP = 4096
EPS = 1e-6
P = 128
C = 512                      # CAP-chunk (columns per pipeline step)
KD, KH = D // P, H // P      # 8 k-subtiles for mm1, 16 for mm2
MT = H // P                  # 16 output row-tiles of mm1 (H rows)
DT = D // P                  # 8 output row-tiles of mm2 (D rows)
NCH = CAP // C               # chunks
S1, S2 = 4, 5                # power-of-2 prescales on W1', W2 (fp8 range fit)

BF16 = mybir.dt.bfloat16
FP8 = mybir.dt.float8e4
F32 = mybir.dt.float32
DR = mybir.MatmulPerfMode.DoubleRow
npbf16 = ml_dtypes.bfloat16
npfp8 = ml_dtypes.float8_e4m3

_nc_cache = {}


def _build(flags):
    """Build the per-core SPMD bass program. flags = (b1_nz, ns_nb_nz, b2_nz).

    The LayerNorm mean over H is folded into the weights on the host
    (W1' = (W1 - rowmean_H(W1)) * 2^S1, b1' = (b1 - mean(b1)) * 2^S1), so
    PSUM holds 2^S1 (h - mu) directly after the W1' matmul."""
    b1_nz, ns_nb_nz, b2_nz = flags
    nc = bacc.Bacc("TRN2", target_bir_lowering=False)

    # x and W1 come in host-relaid per-partition-contiguous layouts: every
    # DMA descriptor covers a 1-4KB contiguous line instead of 128-512B
    # segments (the [D, CAP]/[D, H] layouts cost ~24k descriptors per core,
    # saturating the DMA engines' ~32ns/descriptor processing rate)
    xT_d = nc.dram_tensor("xT", [P, NCH * KD * C], FP8, kind="ExternalInput")
    w1_d = nc.dram_tensor("w1", [P, MT * KD * P], FP8, kind="ExternalInput")
    w2_d = nc.dram_tensor("w2", [H, D], FP8, kind="ExternalInput")
    yT_d = nc.dram_tensor("yT", [D, CAP], F32, kind="ExternalOutput")
    if not ns_nb_nz:
        # fast path: rstd is a per-slot scalar; ship std to the host, which
        # folds 1/std (and b2) into the combine weights
        std_d = nc.dram_tensor("stdr", [1, CAP], F32, kind="ExternalOutput")
    if b1_nz:
        b1_d = nc.dram_tensor("b1", [H, 1], BF16, kind="ExternalInput")
    if ns_nb_nz:
        nsc_d = nc.dram_tensor("nsc", [H, 1], F32, kind="ExternalInput")
        nbs_d = nc.dram_tensor("nbs", [H, 1], F32, kind="ExternalInput")
    if b2_nz and ns_nb_nz:
        b2_d = nc.dram_tensor("b2", [D, 1], F32, kind="ExternalInput")

    xT_r = xT_d.rearrange("p (nch ko c) -> p nch ko c", nch=NCH, ko=KD)
    w1_r = w1_d.rearrange("p (mt ko j) -> p mt ko j", mt=MT, ko=KD)
    w2_r = w2_d.rearrange("(ko p) d -> p ko d", p=P)
    yT_r = yT_d.rearrange("(dt p) c -> p dt c", p=P)

    with tile.TileContext(nc) as tc:
        with (
            tc.tile_pool(name="const", bufs=1) as cpool,
            tc.tile_pool(name="xp", bufs=3) as xpool,
            tc.tile_pool(name="hnp", bufs=2) as hnpool,
            tc.tile_pool(name="sqp", bufs=4) as sqpool,
            tc.tile_pool(name="rows", bufs=3) as rowpool,
            tc.tile_pool(name="rbp", bufs=2) as rbpool,
            tc.tile_pool(name="yp", bufs=3) as ypool,
            tc.tile_pool(name="hgen", bufs=2) as hgenpool,
            tc.tile_pool(name="ps_h", bufs=3, space="PSUM") as ps_h,
            tc.tile_pool(name="ps_y", bufs=4, space="PSUM") as ps_y,
            tc.tile_pool(name="ps_s", bufs=1, space="PSUM") as ps_s,
        ):
            # ---- resident constants (x chunk 0 first, then W1 in row-tile
            # slices so the first matmul group can start after ~0.7MB of DMA,
            # W2 last: not needed until the first mm2) ----------------------
            x_tiles = [None] * NCH

            def emit_x_load(c, split=False):
                x_tiles[c] = xpool.tile([P, KD, C], FP8, tag="x", name="x")
                if split:
                    # chunk 0 gates the first matmul: load in ko-pair strips
                    # (each DoubleRow matmul needs only its own ko pair, so
                    # the first matmul can start after the first strip) on
                    # two issue queues
                    for i, kt in enumerate(range(0, KD, 2)):
                        eng = nc.sync if i % 2 == 0 else nc.scalar
                        eng.dma_start(x_tiles[c][:, kt:kt + 2, :],
                                      xT_r[:, c, kt:kt + 2, :])
                else:
                    for p0 in (0, 64):
                        nc.sync.dma_start(x_tiles[c][p0:p0 + 64, :, :],
                                          xT_r[p0:p0 + 64, c, :, :])

            # interleave the first x chunk with the first W1 row-tile slices
            # so the first matmul group's inputs land as early as possible
            # (w1_sb is mt-major so each slice load is 1KB-contiguous)
            w1_sb = cpool.tile([P, MT, KD, P], FP8, tag="w1", name="w1")
            for p0 in (0, 64):
                nc.sync.dma_start(w1_sb[p0:p0 + 64, 0, :, :],
                                  w1_r[p0:p0 + 64, 0, :, :])
            emit_x_load(0, split=True)
            # warm the PE HAM clock gate with dummy matmuls while the first
            # x/W1 DMAs land, so the first real matmuls run at 2.4 GHz
            warm = cpool.tile([P, 64], BF16, tag="warm", name="warm")
            nc.vector.memset(warm[:], 0.0)
            pwarm = ps_s.tile([P, C], F32, tag="small", name="small")
            for _ in range(48):
                nc.tensor.matmul(pwarm[:64, :64], lhsT=warm[:], rhs=warm[:],
                                 start=True, stop=True, skip_group_check=True)
            # alternate the W1 slice loads across two issue queues so they
            # stay ahead of the mm1 group consuming them
            for mt in range(1, MT):
                eng = nc.sync if mt % 2 == 0 else nc.scalar
                eng.dma_start(w1_sb[:, mt, :, :], w1_r[:, mt, :, :])
            ones_kcol = cpool.tile([P, 1], BF16, tag="ones_kcol", name="ones_kcol")
            nc.vector.memset(ones_kcol[:], 1.0)
            eps_sb = cpool.tile([1, 1], F32, tag="eps", name="eps")
            # Sqrt bias absorbs the power-of-2 prescales (see module docstring)
            if ns_nb_nz:
                nc.vector.memset(eps_sb[:], EPS * 2.0 ** (2 * S1))
            else:
                nc.vector.memset(eps_sb[:], EPS * 2.0 ** (2 * (S1 + S2)))
            if b1_nz:
                b1_sb = cpool.tile([1, H], BF16, tag="b1", name="b1")
                nc.sync.dma_start(b1_sb[:], b1_d.rearrange("h x -> x h"))
                ones_row = cpool.tile([1, C], BF16, tag="ones_row", name="ones_row")
                nc.vector.memset(ones_row[:], 1.0)
            if ns_nb_nz:
                nsc_sb = cpool.tile([P, MT], F32, tag="nsc", name="nsc")
                nc.sync.dma_start(nsc_sb[:], nsc_d.rearrange("(mt p) x -> p mt x", p=P)[:, :, 0])
                nbs_sb = cpool.tile([P, MT], F32, tag="nbs", name="nbs")
                nc.sync.dma_start(nbs_sb[:], nbs_d.rearrange("(mt p) x -> p mt x", p=P)[:, :, 0])
            if b2_nz and ns_nb_nz:
                b2_sb = cpool.tile([P, DT], F32, tag="b2", name="b2")
                nc.sync.dma_start(b2_sb[:], b2_d.rearrange("(dt p) x -> p dt x", p=P)[:, :, 0])
            w2_sb = cpool.tile([P, KH, D], FP8, tag="w2", name="w2")
            for kt in range(KH):
                nc.sync.dma_start(w2_sb[:, kt, :], w2_r[:, kt, :])

            for c in range(NCH):
                xt = x_tiles[c]
                hn = hnpool.tile([P, KH, C], FP8, tag="hn", name="hn")
                hflat = hgenpool.tile([P, KH, C], F32, tag="hflat", name="hflat") if ns_nb_nz else None
                # mm1: 16 row-tile groups of 4 DoubleRow matmuls (256
                # contraction each). ACT squares each PSUM tile (bf16), DVE
                # folds the squared tiles with a linear running sum (one add
                # per group window; after the LAST square only one add
                # remains, so the stats chain launches ~1us after the last
                # mm1 matmul), then PE does a single ones-matmul reduction.
                acc = None
                for mt in range(MT):
                    ph = ps_h.tile([P, C], F32, tag="ph", name="ph")
                    for kt in range(0, KD, 2):
                        nc.tensor.matmul(
                            ph[:], lhsT=w1_sb[:, mt, kt:kt + 2, :],
                            rhs=xt[:, kt:kt + 2, :], start=(kt == 0),
                            stop=(kt == KD - 2 and not b1_nz), perf_mode=DR,
                        )
                    if b1_nz:
                        nc.tensor.matmul(
                            ph[:], lhsT=b1_sb[:, mt * P:(mt + 1) * P], rhs=ones_row[:],
                            start=False, stop=True, skip_group_check=True,
                        )
                    sq = sqpool.tile([P, C], BF16, tag="sq", name="sq")
                    nc.scalar.square(sq[:], ph[:])
                    if ns_nb_nz:
                        nc.vector.tensor_copy(hflat[:, mt, :], ph[:])
                    else:
                        nc.vector.tensor_scalar_max(hn[:, mt, :], ph[:], 0.0)
                    if acc is None:
                        acc = sq
                    else:
                        nxt = sqpool.tile([P, C], BF16, tag=f"acc{mt % 2}",
                                          name="acc")
                        nc.vector.tensor_add(nxt[:], acc[:], sq[:])
                        acc = nxt
                hacc_bf = acc

                if c + 1 < NCH:
                    emit_x_load(c + 1)

                def emit_stats_head(ss, sqrt_scale):
                    # ss[1, C] = sum_p hacc_bf -> std -> rstd (row ops). The
                    # [1, C] DVE reciprocal costs ~3.2us; the whole chain is
                    # issued before the mm2 groups so 4 PSUM y-banks of PE
                    # run-ahead absorb it.
                    nc.tensor.matmul(ss[:1, :], lhsT=ones_kcol[:], rhs=hacc_bf[:],
                                     start=True, stop=True, skip_group_check=True)
                    std = rowpool.tile([1, C], F32, tag="std", name="std")
                    nc.scalar.activation(
                        std[:], ss[:1, :], mybir.ActivationFunctionType.Sqrt,
                        bias=eps_sb[:], scale=sqrt_scale,
                    )
                    rstd = rowpool.tile([1, C], F32, tag="rstd", name="rstd")
                    nc.vector.reciprocal(rstd[:], std[:])
                    return rstd

                def emit_rb(rstd):
                    # broadcast rstd across partitions on the (idle) GpSimd
                    rb = rbpool.tile([P, C], F32, tag="rb", name="rb")
                    nc.gpsimd.partition_broadcast(rb[:], rstd[:], channels=P)
                    return rb

                if ns_nb_nz:
                    # general path: hn = relu(((h-mu)*rstd)*nsc + nbs);
                    # rb here = rstd 2^-S1, and the 2^-S2 is applied at the
                    # mm2 eviction below.
                    ss = ps_s.tile([P, C], F32, tag="small", name="small")
                    rstd = emit_stats_head(ss, 1.0 / H)
                    rb = emit_rb(rstd)
                    for mt in range(MT):
                        tmp = hgenpool.tile([P, C], F32, tag="tmpn", name="tmpn")
                        nc.vector.tensor_mul(tmp[:], hflat[:, mt, :], rb[:])
                        nc.scalar.activation(
                            hn[:, mt, :], tmp[:],
                            mybir.ActivationFunctionType.Relu,
                            bias=nbs_sb[:, mt, None], scale=nsc_sb[:, mt, None],
                        )

                    for dt in range(DT):
                        py = ps_y.tile([P, C], F32, tag="py", name="py")
                        for kt in range(0, KH, 2):
                            nc.tensor.matmul(
                                py[:], lhsT=w2_sb[:, kt:kt + 2, dt * P:(dt + 1) * P],
                                rhs=hn[:, kt:kt + 2, :], start=(kt == 0),
                                stop=(kt == KH - 2), perf_mode=DR,
                            )
                        ysb = ypool.tile([P, C], F32, tag="y", name="y")
                        nc.vector.tensor_scalar_mul(ysb[:], py[:], 2.0 ** (-S2))
                        if b2_nz:
                            nc.vector.tensor_scalar_add(ysb[:], ysb[:], b2_sb[:, dt, None])
                        nc.sync.dma_start(yT_r[:, dt, c * C:(c + 1) * C], ysb[:])
                else:
                    # fast path: the per-slot rstd (and b2) are applied on
                    # the HOST during combine, so mm2 evictions are plain ACT
                    # copies with no dependency on the stats chain. The
                    # device only ships std = sqrt(ss*2^(2S2)/H + eps') rows.
                    pys = [None] * DT

                    def y_mms(dt):
                        pys[dt] = ps_y.tile([P, C], F32, tag="py", name="py")
                        for kt in range(0, KH, 2):
                            nc.tensor.matmul(
                                pys[dt][:], lhsT=w2_sb[:, kt:kt + 2, dt * P:(dt + 1) * P],
                                rhs=hn[:, kt:kt + 2, :], start=(kt == 0),
                                stop=(kt == KH - 2), perf_mode=DR,
                            )

                    def y_evict(dt):
                        ysb = ypool.tile([P, C], F32, tag="y", name="y")
                        nc.vector.tensor_copy(ysb[:], pys[dt][:])
                        if c == NCH - 1 and dt >= DT - 3:
                            # final stores are tail-exposed: halve their
                            # descriptor-serial time and issue from the (now
                            # idle) scalar queue
                            for p0 in (0, 64):
                                nc.scalar.dma_start(
                                    yT_r[p0:p0 + 64, dt, c * C:(c + 1) * C],
                                    ysb[p0:p0 + 64, :])
                        else:
                            nc.sync.dma_start(yT_r[:, dt, c * C:(c + 1) * C], ysb[:])

                    # stats ones-matmul sits in the PE's in-order stream:
                    # issue it after the first mm2 group so the PE never
                    # waits on the square/add chain draining
                    y_mms(0)
                    ss = ps_s.tile([P, C], F32, tag="small", name="small")
                    nc.tensor.matmul(ss[:1, :], lhsT=ones_kcol[:], rhs=hacc_bf[:],
                                     start=True, stop=True, skip_group_check=True)
                    std = rowpool.tile([1, C], F32, tag="std", name="std")
                    nc.scalar.activation(
                        std[:], ss[:1, :], mybir.ActivationFunctionType.Sqrt,
                        bias=eps_sb[:], scale=2.0 ** (2 * S2) / H,
                    )
                    nc.sync.dma_start(std_d[:, c * C:(c + 1) * C], std[:])
                    y_evict(0)
                    for dt in range(1, DT):
                        y_mms(dt)
                        y_evict(dt)

    nc.compile()
    return nc


# ------------------------------------------------------------ host logic ---
def _route(x0, ln_scale, ln_bias, Wr, br):
    """LayerNorm -> relu -> router logits -> top-2 -> gates (float64 math)."""
    x = x0.astype(np.float64)
    mu = x.mean(axis=-1, keepdims=True)
    var = np.square(x - mu).mean(axis=-1, keepdims=True)
    xn = (x - mu) / np.sqrt(var + EPS)
    xn = xn * ln_scale.astype(np.float64) + ln_bias.astype(np.float64)
    np.maximum(xn, 0.0, out=xn)
    logits = xn @ Wr.astype(np.float64) + br.astype(np.float64)

    n = logits.shape[0]
    rows = np.arange(n)
    i0 = np.argmax(logits, axis=1)
    l0 = logits[rows, i0]
    tmp = logits.copy()
    tmp[rows, i0] = -np.inf
    i1 = np.argmax(tmp, axis=1)
    l1 = tmp[rows, i1]
    # softmax over (l0, l1); l0 >= l1
    e1 = np.exp(l1 - l0)
    g0 = 1.0 / (1.0 + e1)
    g1 = e1 / (1.0 + e1)
    top_idx = np.stack([i0, i1], axis=1).astype(np.int64)
    gates = np.stack([g0, g1], axis=1)
    return xn.astype(np.float32), top_idx, gates


def _positions(top_idx):
    """Capacity positions: running per-expert count in token-major slot order."""
    eidx = top_idx.reshape(-1)
    nk = eidx.shape[0]
    oh = (eidx[:, None] == np.arange(E)[None, :]).astype(np.int64)
    pos = np.cumsum(oh, axis=0)[np.arange(nk), eidx] - 1
    mask = pos < CAP
    pos_c = np.minimum(pos, CAP - 1)
    return eidx, pos, pos_c, mask


def kernel(**inputs):
    x0 = np.asarray(inputs["x0"], np.float32)
    ln_scale = np.asarray(inputs["ln_scale"], np.float32)
    ln_bias = np.asarray(inputs["ln_bias"], np.float32)
    Wr = np.asarray(inputs["Wr"], np.float32)
    br = np.asarray(inputs["br"], np.float32)
    W1 = np.asarray(inputs["W1"], np.float32)
    b1 = np.asarray(inputs["b1"], np.float32)
    n_scale = np.asarray(inputs["n_scale"], np.float32)
    n_bias = np.asarray(inputs["n_bias"], np.float32)
    W2 = np.asarray(inputs["W2"], np.float32)
    b2 = np.asarray(inputs["b2"], np.float32)

    # ---- host routing + dispatch ---------------------------------------
    xn, top_idx, gates = _route(x0, ln_scale, ln_bias, Wr, br)
    eidx, pos, pos_c, mask = _positions(top_idx)

    tok_of_slot = np.repeat(np.arange(N), TOPK)
    keep = mask
    expert_inputs = np.zeros((E, CAP, D), np.float32)
    expert_inputs[eidx[keep], pos[keep]] = xn[tok_of_slot[keep]]

    # ---- build / fetch compiled program --------------------------------
    b1_nz = bool(np.any(b1))
    ns_nb_nz = bool(np.any(n_scale != 1.0) or np.any(n_bias))
    b2_nz = bool(np.any(b2))
    flags = (b1_nz, ns_nb_nz, b2_nz)
    if flags not in _nc_cache:
        _nc_cache[flags] = _build(flags)
    nc = _nc_cache[flags]

    # ---- per-core inputs ----------------------------------------------
    in_maps = []
    for e in range(E):
        # Fold the LayerNorm mean over H into the weights: x @ W1' = h - mu.
        w1p = W1[e].astype(np.float64)
        w1p = w1p - w1p.mean(axis=1, keepdims=True)
        # per-partition-contiguous relayouts (see _build): x [P, NCH*KD*C],
        # W1 [P, MT*KD*128]
        xT8 = expert_inputs[e].T.astype(npfp8)            # [D, CAP]
        x_lay = np.ascontiguousarray(
            xT8.reshape(KD, P, NCH, C).transpose(1, 2, 0, 3).reshape(P, -1))
        w1q = (w1p * 2.0 ** S1).astype(npfp8)             # [D, H]
        w1_lay = np.ascontiguousarray(
            w1q.reshape(KD, P, MT, P).transpose(1, 2, 0, 3).reshape(P, -1))
        m = {
            "xT": x_lay,
            "w1": w1_lay,
            "w2": (W2[e].astype(np.float64) * 2.0 ** S2).astype(npfp8),
        }
        if b1_nz:
            b1p = b1[e].astype(np.float64)
            b1p = b1p - b1p.mean()
            m["b1"] = (b1p * 2.0 ** S1).astype(npbf16)[:, None]
        if ns_nb_nz:
            m["nsc"] = n_scale[e].astype(np.float32)[:, None]
            m["nbs"] = n_bias[e].astype(np.float32)[:, None]
        if b2_nz and ns_nb_nz:
            m["b2"] = b2[e].astype(np.float32)[:, None]
        in_maps.append(m)

    res = run_bass_kernel_spmd(nc, in_maps, core_ids=list(range(E)))

    # ---- combine -------------------------------------------------------
    yT_all = np.stack([res.results[e]["yT"] for e in range(E)])  # [E, D, CAP]
    w = (gates.astype(np.float32) * mask.reshape(N, TOPK))
    pos2 = pos_c.reshape(N, TOPK)
    mix = np.zeros((N, D), np.float32)
    if ns_nb_nz:
        for k in range(TOPK):
            mix += yT_all[top_idx[:, k], :, pos2[:, k]] * w[:, k:k + 1]
    else:
        # fast path: device output is raw 2^(S1+S2) W2^T relu(h-mu); fold
        # rstd 2^-(S1+S2) = 1/std (and b2) into the per-slot combine weights
        std_all = np.stack([np.asarray(res.results[e]["stdr"], np.float64)
                            .reshape(CAP) for e in range(E)])  # [E, CAP]
        rb_all = 1.0 / std_all
        for k in range(TOPK):
            wk = (w[:, k] * rb_all[top_idx[:, k], pos2[:, k]]).astype(np.float32)
            mix += yT_all[top_idx[:, k], :, pos2[:, k]] * wk[:, None]
            if b2_nz:
                mix += b2[top_idx[:, k]] * w[:, k:k + 1]
    return x0 + mix
